# revision 1
# baseline (speedup 1.0000x reference)
"""LocalAttention (B=4, H=16, L=2048, D=64, R=256) Trainium2 kernel.

The reference mask `(j-i >= 2048) | (j-i <= 1792)` keeps only keys with
j - i >= 1793.  Consequences (verified numerically vs the reference):
  * queries i in [0, 254] attend to the key band j in [i+1793, 2047]
    (masked logits underflow to exactly 0 after exp in f32, like the
    reference's exp(-10000 - max)),
  * queries i in [255, 2047] have every key masked -> softmax is uniform
    -> output row = mean(v over L).

So per (b, h) head we compute:
  1. mean_v = (1/2048) * sum_l v[l, :]            -> rows 255..2047
  2. a 255x255 "triangular band" attention with
     Q = q[0:255], K = k[1793:2047], V = v[1793:2047]  -> rows 0..254

Sharding: 64 (b,h) pairs, 8 per NeuronCore (data+head parallel, no
cross-device comm).  Per core the host ships: transposed Q/K bands
(qkT), the V band with fused ones-columns (vbo, for the softmax
denominator), and the full v (for the mean).  Host work is layout
marshalling only (transpose/concat), no arithmetic.

DMA queues are spread across the three issue engines (SP-HWDGE,
ACT-HWDGE, gpsimd-SWDGE) since DMA is the critical path.

NOTE this walrus build rejects instructions with more than one attached
sync wait, so `_legalize_waits` splits them into single-wait NoOps.
"""

import numpy as np
from contextlib import ExitStack

import concourse.bass as bass
import concourse.mybir as mybir
import concourse.tile as tile
from concourse.bass_utils import run_bass_kernel_spmd

B, H, L, D = 4, 16, 2048, 64
BH = B * H            # 64 (b,h) pairs
NCORES = 8
PER = BH // NCORES    # 8 pairs per core
BAND = 256            # padded band (queries 0..255 / keys 1792..2047)
NQ = 255              # valid band queries (0..254)
JCH = 14              # non-band v rows packed per partition (1792/128)

F32 = mybir.dt.float32
EXP = mybir.ActivationFunctionType.Exp
SCALE = 0.125         # 1/sqrt(D)


def _build_bass():
    nc = bass.Bass()
    qkT = nc.declare_dram_parameter("qkT", [PER, D, 2 * BAND], F32, isOutput=False)
    vbo = nc.declare_dram_parameter("vbo", [PER, 128, 2 * (D + 1)], F32,
                                    isOutput=False)
    # v rows 0:1792 in j-major layout: vm[p, d*14+j] = v[14p+j, d] (host
    # marshalled) so the per-d reduce over j is unit-stride on DVE; the
    # band rows 1792:2048 reach the mean through vbo instead
    vv = nc.declare_dram_parameter("vm", [PER, 128, JCH * D], F32, isOutput=False)
    out = nc.declare_dram_parameter("out", [PER, L, D], F32, isOutput=True)

    with tile.TileContext(nc) as tc:
        with ExitStack() as ctx:
            vpool = ctx.enter_context(tc.tile_pool(name="vpool", bufs=3))
            io = ctx.enter_context(tc.tile_pool(name="io", bufs=3))
            ep = ctx.enter_context(tc.tile_pool(name="ep", bufs=3))
            small = ctx.enter_context(tc.tile_pool(name="small", bufs=4))
            ps_st = ctx.enter_context(tc.tile_pool(name="ps_st", bufs=3, space="PSUM"))
            ps_u = ctx.enter_context(tc.tile_pool(name="ps_u", bufs=4, space="PSUM"))

            for ibh in range(PER):
                # ---------------- loads ----------------
                # full v, contiguous 512KB (partition p = rows 16p..16p+15),
                # on the SP HWDGE queue
                v_tile = vpool.tile([128, JCH * D], F32)
                nc.sync.dma_start(out=v_tile, in_=vv[ibh])
                # transposed Q|K band [D, 512] on the ACT HWDGE queue,
                # V band + ones [128, 130] on the SWDGE queue
                qk = io.tile([D, 2 * BAND], F32, tag="qk")
                nc.gpsimd.dma_start(out=qk, in_=qkT[ibh])
                vb = io.tile([128, 2 * (D + 1)], F32, tag="vb")
                nc.scalar.dma_start(out=vb, in_=vbo[ibh])

                # ---------------- mean(v) over L ----------------
                vsum = small.tile([128, D], F32)
                nc.vector.reduce_sum(
                    out=vsum[:, :, None],
                    in_=v_tile.rearrange("p (d j) -> p d j", j=JCH),
                    axis=mybir.AxisListType.X,
                )
                mean_ps = ps_u.tile([1, D], F32, tag="u")
                ones_col = vb[:, D:D + 1]
                nc.tensor.matmul(mean_ps, lhsT=ones_col, rhs=vsum,
                                 start=True, stop=False)
                nc.tensor.matmul(mean_ps, lhsT=ones_col, rhs=vb[:, 0:D],
                                 start=False, stop=False)
                nc.tensor.matmul(mean_ps, lhsT=ones_col,
                                 rhs=vb[:, D + 1:2 * D + 1],
                                 start=False, stop=True)
                mean_sb = small.tile([1, D], F32)
                nc.vector.tensor_scalar_mul(mean_sb, mean_ps, 1.0 / float(L))
                # broadcast mean row to out rows 255..2047 (replicated source)
                msb = mean_sb[:, :]
                mean_bc = bass.AP(
                    tensor=msb.tensor,
                    offset=msb.offset,
                    ap=[list(msb.ap[0]), [0, L - NQ], [1, D]],
                )
                nc.gpsimd.dma_start(out=out[ibh, NQ:L, :], in_=mean_bc)

                # ---------------- band attention ----------------
                # scores (keys on partitions, queries on free dim), both
                # key-chunks into one PSUM tile: cols 0:128 = (k0, q0),
                # cols 128:384 = (k1, q0|q1)
                st = ps_st.tile([128, 384], F32, tag="st")
                nc.tensor.matmul(st[:, 0:128], lhsT=qk[:, BAND:BAND + 128],
                                 rhs=qk[:, 0:128], start=True, stop=True)
                nc.tensor.matmul(st[:, 128:384], lhsT=qk[:, BAND + 128:2 * BAND],
                                 rhs=qk[:, 0:BAND], start=True, stop=True)

                # exp(score/sqrt(D)); no max-subtraction needed (|s| <= ~7)
                e = ep.tile([128, 384], F32)
                nc.scalar.activation(e, st, EXP, scale=SCALE)
                # mask on the idle gpsimd engine: zero the invalid entries
                # key chunk0 vs q chunk0: keep iff p - f - 1 >= 0 (f < p)
                nc.gpsimd.affine_select(
                    out=e[:, 0:128], in_=e[:, 0:128],
                    compare_op=mybir.AluOpType.is_ge,
                    fill=0.0, base=-1, channel_multiplier=1,
                    pattern=[[-1, 128]],
                )
                # key chunk1 vs q0|q1: keep iff p - f + 127 >= 0
                nc.gpsimd.affine_select(
                    out=e[:, 128:384], in_=e[:, 128:384],
                    compare_op=mybir.AluOpType.is_ge,
                    fill=0.0, base=127, channel_multiplier=1,
                    pattern=[[-1, BAND]],
                )

                # U = P^T V (+ denominator in column D via the ones column)
                u0 = ps_u.tile([128, D + 1], F32, tag="u")
                nc.tensor.matmul(u0, lhsT=e[:, 0:128], rhs=vb[:, 0:D + 1],
                                 start=True, stop=False)
                nc.tensor.matmul(u0, lhsT=e[:, 128:256], rhs=vb[:, D + 1:],
                                 start=False, stop=True)
                u1 = ps_u.tile([128, D + 1], F32, tag="u")
                nc.tensor.matmul(u1, lhsT=e[:, 256:384], rhs=vb[:, D + 1:],
                                 start=True, stop=True)

                # normalize rows and store the band output
                r0 = small.tile([128, 1], F32, tag="r")
                r1 = small.tile([128, 1], F32, tag="r")
                nc.vector.reciprocal(r0, u0[:, D:D + 1])
                # query row 255 (f=127 of chunk1) is fully masked -> den = 0;
                # keep it finite (the row is never stored)
                den1 = small.tile([128, 1], F32, tag="r")
                nc.vector.tensor_scalar_add(den1, u1[:, D:D + 1], 1e-20)
                nc.vector.reciprocal(r1, den1)
                ob0 = small.tile([128, D], F32, tag="ob")
                ob1 = small.tile([128, D], F32, tag="ob")
                nc.vector.tensor_scalar_mul(ob0, u0[:, 0:D], r0)
                nc.vector.tensor_scalar_mul(ob1, u1[:, 0:D], r1)
                nc.sync.dma_start(out=out[ibh, 0:128, :], in_=ob0)
                nc.scalar.dma_start(out=out[ibh, 128:NQ, :], in_=ob1[0:127, :])

    return nc


def _legalize_waits(nc):
    """This walrus build rejects instructions carrying more than one
    attached sync wait (per-struct slot limits, e.g. PE Matmult and the
    kernel-tail Drain).  Split every multi-wait instruction's waits into
    preceding single-wait NoOps on the same engine queue — same-queue
    ordering preserves semantics exactly."""
    n = 0
    for fn in nc.m.functions:
        for blk in fn.blocks:
            new_insts = []
            for inst in blk.instructions:
                si = inst.sync_info
                if si is not None and si.on_wait and len(si.on_wait) > 1:
                    for w in si.on_wait:
                        n += 1
                        new_insts.append(mybir.InstNoOp(
                            name=f"legwait-{n}",
                            engine=inst.engine,
                            ins=[], outs=[],
                            sync_info=mybir.SyncInfo(on_wait=[w], on_update=[]),
                            bass_nofuse=True,
                        ))
                    inst.sync_info = mybir.SyncInfo(
                        on_wait=[], on_update=list(si.on_update or []))
                new_insts.append(inst)
            blk.instructions[:] = new_insts


_NC = None
_LEGALIZED = False


def _get_nc(legalize=False):
    global _NC, _LEGALIZED
    if _NC is None:
        _NC = _build_bass()
    if legalize and not _LEGALIZED:
        # CoreSim chokes on the injected NoOps, so only legalize for the
        # HW compile path
        _legalize_waits(_NC)
        _LEGALIZED = True
    return _NC


def _make_in_maps(q, k, v):
    qf = np.asarray(q, dtype=np.float32).reshape(BH, L, D)
    kf = np.asarray(k, dtype=np.float32).reshape(BH, L, D)
    vf = np.asarray(v, dtype=np.float32).reshape(BH, L, D)
    # host-side layout marshalling (no arithmetic): transpose the Q/K
    # bands, pack the V band with ones-columns
    qkT = np.concatenate(
        [qf[:, 0:BAND, :].transpose(0, 2, 1),
         kf[:, L - BAND:L, :].transpose(0, 2, 1)], axis=2)
    qkT = np.ascontiguousarray(qkT)                      # [BH, D, 512]
    vband = vf[:, L - BAND:L, :].reshape(BH, 2, 128, D)  # [BH, 2, 128, 64]
    vbo = np.ones((BH, 128, 2 * (D + 1)), dtype=np.float32)
    vbo[:, :, 0:D] = vband[:, 0]
    vbo[:, :, D + 1:2 * D + 1] = vband[:, 1]
    in_maps = []
    for c in range(NCORES):
        s = slice(c * PER, (c + 1) * PER)
        in_maps.append({
            "qkT": qkT[s],
            "vbo": np.ascontiguousarray(vbo[s]),
            "vm": np.ascontiguousarray(
                vf[s, 0:128 * JCH].reshape(PER, 128, JCH, D)
                .transpose(0, 1, 3, 2).reshape(PER, 128, JCH * D)),
        })
    return in_maps


def _run(q, k, v, **kwargs):
    nc = _get_nc(legalize=True)
    in_maps = _make_in_maps(q, k, v)
    return run_bass_kernel_spmd(nc, in_maps, list(range(NCORES)), **kwargs)


def kernel(q, k, v):
    res = _run(q, k, v)
    outs = [res.results[c]["out"] for c in range(NCORES)]
    return np.concatenate(outs, axis=0).reshape(B, H, L, D)



# revision 4
# speedup vs baseline: 1.4344x; 1.4344x over previous
"""LocalAttention (B=4, H=16, L=2048, D=64, R=256) Trainium2 kernel, v3.

Math (identical to the verified baseline): queries 0..254 attend keys
j in [i+1793, 2047]; queries 255..2047 output mean(v over L).

Design notes (from the measured CoreSim cost model):
  * fp32r matmuls (bit-identical to f32 in the interpreter; 1 cycle/row
    when out free-size >= 256).
  * mean(v) via 16 one-column PE matmuls per head into a [64, 8] PSUM
    tile; transposed by one identity matmul.  No big DVE reduce.
  * masks as mask-tile multiplies on the otherwise-idle DVE.
  * only 3 DMA queues exist (SP/ACT HWDGE + Pool SWDGE); loads are
    round-robined and issued first, the ACT exp-table warmup runs inside
    the initial DMA-latency dead window.
  * stores: band rows as 4-head-group DMAs that skip row 255; mean rows
    via seed stores (rows 255+2047) + DRAM->DRAM broadcasts of rows
    256..2046 on the same queue (ordered), split in two halves so the
    first half overlaps compute.
"""

import numpy as np
from contextlib import ExitStack

import concourse.bass as bass
import concourse.mybir as mybir
import concourse.tile as tile
from concourse.bass_utils import run_bass_kernel_spmd

B, H, L, D = 4, 16, 2048, 64
BH = B * H
NCORES = 8
PER = BH // NCORES     # 8 heads per core
BAND = 256             # band keys 1792..2047
NQ = 255               # valid band queries 0..254
NBC = 7                # non-band 256-row chunks (7*256 = 1792 rows)
VW = NBC * 2 * D + 2 * D   # 1024 floats per partition per head

F32 = mybir.dt.float32
F32R = mybir.dt.float32r
EXP = mybir.ActivationFunctionType.Exp
SCALE = 0.125          # 1/sqrt(D)
INVL = 1.0 / float(L)  # 2^-11, exact


def _build_bass():
    nc = bass.Bass()
    # qk: [128, 4*512]; heads even on partitions 0:64, odd on 64:128;
    # per head-slot 512 cols = [qT(256) | k0T(128) | k1T(128)]
    qk = nc.declare_dram_parameter("qk", [128, 4 * 512], F32R, isOutput=False)
    # vv: per head [128, 1024]: cols 0:896 non-band rows in pair layout
    # (c7, e2, d), cols 896:1024 band rows key-on-partition (c2, d)
    vv = nc.declare_dram_parameter("vv", [PER, 128, VW], F32, isOutput=False)
    out = nc.declare_dram_parameter("out", [PER, L, D], F32, isOutput=True)

    with tile.TileContext(nc) as tc:
        with ExitStack() as ctx:
            const = ctx.enter_context(tc.tile_pool(name="const", bufs=1))
            vpool = ctx.enter_context(tc.tile_pool(name="vpool", bufs=PER))
            io = ctx.enter_context(tc.tile_pool(name="io", bufs=1))
            ep = ctx.enter_context(tc.tile_pool(name="ep", bufs=3))
            sm = ctx.enter_context(tc.tile_pool(name="sm", bufs=8))
            ps_st = ctx.enter_context(tc.tile_pool(name="ps_st", bufs=3,
                                                   space="PSUM"))
            ps_u = ctx.enter_context(tc.tile_pool(name="ps_u", bufs=3,
                                                  space="PSUM"))
            ps_m = ctx.enter_context(tc.tile_pool(name="ps_m", bufs=1,
                                                  space="PSUM"))

            # ---- constants (Pool, first so the ACT warmup can start) ----
            scrap = const.tile([1, 2], F32, tag="scrap")
            nc.gpsimd.memset(scrap, 0.0)
            ones = const.tile([128, 1], F32, tag="ones")
            nc.gpsimd.memset(ones, 1.0)
            lones = const.tile([128, 1], F32, tag="lones")
            nc.gpsimd.memset(lones, INVL)
            ident = const.tile([64, 64], F32, tag="ident")
            nc.gpsimd.memset(ident, 1.0)
            nc.gpsimd.affine_select(
                out=ident, in_=ident, compare_op=mybir.AluOpType.is_equal,
                fill=0.0, base=0, channel_multiplier=1, pattern=[[-1, 64]],
            )
            # mask[p, f] = 1 iff p >= f + 1 (strict lower triangle)
            mask = const.tile([128, 128], F32, tag="mask")
            nc.gpsimd.memset(mask, 1.0)
            nc.gpsimd.affine_select(
                out=mask, in_=mask, compare_op=mybir.AluOpType.is_ge,
                fill=0.0, base=-1, channel_multiplier=1, pattern=[[-1, 128]],
            )
            # ACT exp-table warmup inside the DMA-latency dead window
            warm = const.tile([1, 2], F32, tag="warm")
            nc.scalar.activation(warm, scrap, EXP)

            # ---- loads (qk first; vv halves round-robined) ----
            qkt = io.tile([128, 4 * 512], F32R, tag="qkt")
            nc.sync.dma_start(out=qkt[:, 0:512], in_=qk[:, 0:512])
            nc.gpsimd.dma_start(out=qkt[:, 512:1024], in_=qk[:, 512:1024])
            nc.sync.dma_start(out=qkt[:, 1024:1536], in_=qk[:, 1024:1536])
            nc.gpsimd.dma_start(out=qkt[:, 1536:2048], in_=qk[:, 1536:2048])
            vts = []
            for h in range(PER):
                vt = vpool.tile([128, VW], F32, tag="vt")
                vts.append(vt)
            # (head, half) -> queue; SP x7, Pool x6, ACT x3; A-half (h0-3)
            # and B-half (h4-7) interleaved so both mean halves gate early
            HALF_Q = [
                (0, 0, "sp"), (0, 1, "pool"), (1, 0, "act"),
                (1, 1, "sp"), (2, 0, "pool"), (2, 1, "sp"),
                (3, 0, "pool"), (3, 1, "act"), (4, 0, "sp"),
                (4, 1, "pool"), (5, 0, "sp"), (5, 1, "sp"),
                (6, 0, "pool"), (6, 1, "sp"), (7, 0, "pool"),
                (7, 1, "act"),
            ]
            ENG = {"sp": nc.sync, "act": nc.scalar, "pool": nc.gpsimd}
            for h, half, qn in HALF_Q:
                sl = slice(half * 512, (half + 1) * 512)
                ENG[qn].dma_start(out=vts[h][:, sl], in_=vv[h, :, sl])

            # ---- per-head compute ----
            mean_ps = ps_m.tile([64, PER], F32, tag="mt")
            ob = io.tile([128, PER * 2 * D], F32, tag="ob")

            for h in range(PER):
                vt = vts[h]
                v0 = vt[:, 896:896 + D]
                v1 = vt[:, 896 + D:896 + 2 * D]

                # mean chain: 16 one-column matmuls into mean_ps[:, h]
                mcol = mean_ps[:, h:h + 1]
                for s in range(NBC * 2):
                    nc.tensor.matmul(mcol, lhsT=vt[:, s * D:(s + 1) * D],
                                     rhs=lones, start=(s == 0), stop=False)
                nc.tensor.matmul(mcol, lhsT=v0, rhs=lones,
                                 start=False, stop=False)
                nc.tensor.matmul(mcol, lhsT=v1, rhs=lones,
                                 start=False, stop=True)

                base_p = (h % 2) * 64
                col0 = (h // 2) * 512
                qT = qkt[base_p:base_p + 64, col0:col0 + 256]
                k0 = qkt[base_p:base_p + 64, col0 + 256:col0 + 384]
                k1 = qkt[base_p:base_p + 64, col0 + 384:col0 + 512]

                # scores: keys on partitions; cols = [c0 x q0:128 | c1 x q0:256]
                st = ps_st.tile([128, 384], F32, tag="st")
                nc.tensor.matmul(st[:, 0:128], lhsT=k0, rhs=qT[:, 0:128],
                                 start=True, stop=True)
                nc.tensor.matmul(st[:, 128:384], lhsT=k1, rhs=qT,
                                 start=True, stop=True)

                # exp (|s|/8 <= ~6, no max subtraction needed)
                e = ep.tile([128, 384], F32, tag="e")
                nc.scalar.activation(e, st, EXP, scale=SCALE)
                # masks: strict-lower-triangle; DVE for early heads, Pool
                # (idle after its loads) for late heads
                if h < 6:
                    nc.vector.tensor_tensor(
                        out=e[:, 0:128], in0=e[:, 0:128], in1=mask,
                        op=mybir.AluOpType.mult)
                    nc.vector.tensor_tensor(
                        out=e[:, 256:384], in0=e[:, 256:384], in1=mask,
                        op=mybir.AluOpType.mult)
                else:
                    for sl in (slice(0, 128), slice(256, 384)):
                        nc.gpsimd.affine_select(
                            out=e[:, sl], in_=e[:, sl],
                            compare_op=mybir.AluOpType.is_ge,
                            fill=0.0, base=-1, channel_multiplier=1,
                            pattern=[[-1, 128]],
                        )

                # U = P^T V, den = P^T 1
                u0 = ps_u.tile([128, D + 1], F32, tag="u")
                nc.tensor.matmul(u0[:, 0:D], lhsT=e[:, 0:128], rhs=v0,
                                 start=True, stop=False)
                nc.tensor.matmul(u0[:, 0:D], lhsT=e[:, 128:256], rhs=v1,
                                 start=False, stop=True)
                nc.tensor.matmul(u0[:, D:D + 1], lhsT=e[:, 0:128], rhs=ones,
                                 start=True, stop=False)
                nc.tensor.matmul(u0[:, D:D + 1], lhsT=e[:, 128:256], rhs=ones,
                                 start=False, stop=True)
                u1 = ps_u.tile([128, D + 1], F32, tag="u")
                nc.tensor.matmul(u1[:, 0:D], lhsT=e[:, 256:384], rhs=v1,
                                 start=True, stop=True)
                nc.tensor.matmul(u1[:, D:D + 1], lhsT=e[:, 256:384], rhs=ones,
                                 start=True, stop=True)

                # normalize on DVE (recips free, muls 192)
                r0 = sm.tile([128, 1], F32, tag="r")
                r1 = sm.tile([128, 1], F32, tag="r")
                den1 = sm.tile([128, 1], F32, tag="r")
                nc.vector.reciprocal(r0, u0[:, D:D + 1])
                # query 255 (u1 row 127) is fully masked -> den 0; keep finite
                nc.vector.tensor_scalar_add(den1, u1[:, D:D + 1], 1e-30)
                nc.vector.reciprocal(r1, den1)
                nc.vector.tensor_scalar_mul(
                    ob[:, (2 * h) * D:(2 * h + 1) * D], u0[:, 0:D], r0)
                nc.vector.tensor_scalar_mul(
                    ob[:, (2 * h + 1) * D:(2 * h + 2) * D], u1[:, 0:D], r1)

                if h == 3 or h == 7:
                    # band stores for the finished 4-head group: rows 0..127
                    # (c0) and 128..254 (c1, partitions 0..126); row 255 is
                    # left to the mean path
                    g0 = h - 3
                    src = ob.rearrange("p (h c d) -> p h c d", h=PER, d=D)
                    inA = bass.AP(
                        tensor=src.tensor, offset=src.offset + g0 * 2 * D,
                        ap=[list(src.ap[0]), [2 * D, 4], [1, D]],
                    )
                    dstA = bass.AP(
                        tensor=out, offset=g0 * L * D,
                        ap=[[D, 128], [L * D, 4], [1, D]],
                    )
                    inB = bass.AP(
                        tensor=src.tensor,
                        offset=src.offset + g0 * 2 * D + D,
                        ap=[[src.ap[0][0], 127], [2 * D, 4], [1, D]],
                    )
                    dstB = bass.AP(
                        tensor=out, offset=g0 * L * D + 128 * D,
                        ap=[[D, 127], [L * D, 4], [1, D]],
                    )
                    engA = nc.sync
                    engB = nc.sync if h == 3 else nc.scalar
                    engA.dma_start(out=dstA, in_=inA)
                    engB.dma_start(out=dstB, in_=inB)

                if h == 3 or h == 7:
                    # mean seed + broadcast for the finished half; copies on
                    # Pool (DVE is busy with norms by now)
                    g0 = h - 3
                    mt_sb = sm.tile([64, 4], F32, tag="mt_sb")
                    nc.vector.tensor_scalar_mul(mt_sb, mean_ps[:, g0:g0 + 4],
                                                1.0)
                    mrow_ps = ps_m.tile([4, 64], F32, tag="mrow")
                    nc.tensor.matmul(mrow_ps, lhsT=mt_sb, rhs=ident,
                                     start=True, stop=True)
                    mrow = sm.tile([4, 64], F32, tag="mrow_sb")
                    nc.vector.tensor_scalar_mul(mrow, mrow_ps, 1.0)
                    # seed rows 255 and 2047 of the 4 heads (one DMA, 500)
                    seed_in = bass.AP(
                        tensor=mrow.tensor, offset=mrow.offset,
                        ap=[list(mrow.ap[0]), [0, 2], [1, D]],
                    )
                    seed_out = bass.AP(
                        tensor=out, offset=g0 * L * D + NQ * D,
                        ap=[[L * D, 4], [(L - 1 - NQ) * D, 2], [1, D]],
                    )
                    # DRAM->DRAM broadcast rows 256..2046 from row 2047,
                    # same queue as the seed => ordered after it
                    bc_in = bass.AP(
                        tensor=out, offset=g0 * L * D + (L - 1) * D,
                        ap=[[0, L - 2 - NQ], [L * D, 4], [1, D]],
                    )
                    bc_out = bass.AP(
                        tensor=out, offset=g0 * L * D + (NQ + 1) * D,
                        ap=[[D, L - 2 - NQ], [L * D, 4], [1, D]],
                    )
                    engM = nc.gpsimd
                    engM.dma_start(out=seed_out, in_=seed_in)
                    engM.dma_start(out=bc_out, in_=bc_in)

    return nc


def _legalize_waits(nc):
    """This walrus build rejects instructions carrying more than one attached
    sync wait: split them into preceding single-wait NoOps (same queue =>
    same semantics)."""
    n = 0
    for fn in nc.m.functions:
        for blk in fn.blocks:
            new_insts = []
            for inst in blk.instructions:
                si = inst.sync_info
                if si is not None and si.on_wait and len(si.on_wait) > 1:
                    for w in si.on_wait:
                        n += 1
                        new_insts.append(mybir.InstNoOp(
                            name=f"legwait-{n}",
                            engine=inst.engine,
                            ins=[], outs=[],
                            sync_info=mybir.SyncInfo(on_wait=[w], on_update=[]),
                            bass_nofuse=True,
                        ))
                    inst.sync_info = mybir.SyncInfo(
                        on_wait=[], on_update=list(si.on_update or []))
                new_insts.append(inst)
            blk.instructions[:] = new_insts


_NC = None
_LEGALIZED = False


def _get_nc(legalize=False):
    global _NC, _LEGALIZED
    if _NC is None:
        _NC = _build_bass()
    if legalize and not _LEGALIZED:
        _legalize_waits(_NC)
        _LEGALIZED = True
    return _NC


def _make_in_maps(q, k, v):
    qf = np.asarray(q, dtype=np.float32).reshape(BH, L, D)
    kf = np.asarray(k, dtype=np.float32).reshape(BH, L, D)
    vf = np.asarray(v, dtype=np.float32).reshape(BH, L, D)
    qT = qf[:, 0:BAND, :].transpose(0, 2, 1)          # [BH, 64, 256]
    kT = kf[:, L - BAND:L, :].transpose(0, 2, 1)      # [BH, 64, 256]
    qkcat = np.concatenate([qT, kT], axis=2)          # [BH, 64, 512]
    # vv: non-band pair layout cols 0:896, band key-on-partition cols 896:1024
    vvh = np.empty((BH, 128, VW), np.float32)
    vvh[:, :, 0:NBC * 2 * D] = (
        vf[:, 0:NBC * 256].reshape(BH, NBC, 128, 2, D)
        .transpose(0, 2, 1, 3, 4).reshape(BH, 128, NBC * 2 * D))
    vvh[:, :, NBC * 2 * D:] = (
        vf[:, L - BAND:L].reshape(BH, 2, 128, D)
        .transpose(0, 2, 1, 3).reshape(BH, 128, 2 * D))
    in_maps = []
    for c in range(NCORES):
        s = slice(c * PER, (c + 1) * PER)
        qkc = qkcat[s]                                 # [8, 64, 512]
        qkp = np.empty((128, 4 * 512), np.float32)
        qkp[0:64] = qkc[0::2].transpose(1, 0, 2).reshape(64, 4 * 512)
        qkp[64:128] = qkc[1::2].transpose(1, 0, 2).reshape(64, 4 * 512)
        in_maps.append({
            "qk": np.ascontiguousarray(qkp),
            "vv": np.ascontiguousarray(vvh[s]),
        })
    return in_maps


def _run(q, k, v, **kwargs):
    nc = _get_nc(legalize=True)
    in_maps = _make_in_maps(q, k, v)
    return run_bass_kernel_spmd(nc, in_maps, list(range(NCORES)), **kwargs)


def kernel(q, k, v):
    res = _run(q, k, v)
    outs = [res.results[c]["out"] for c in range(NCORES)]
    return np.concatenate(outs, axis=0).reshape(B, H, L, D)


# revision 5
# speedup vs baseline: 1.5487x; 1.0797x over previous
"""LocalAttention (B=4, H=16, L=2048, D=64, R=256) Trainium2 kernel, v3.

Math (identical to the verified baseline): queries 0..254 attend keys
j in [i+1793, 2047]; queries 255..2047 output mean(v over L).

Design notes (from the measured CoreSim cost model):
  * fp32r matmuls (bit-identical to f32 in the interpreter; 1 cycle/row
    when out free-size >= 256).
  * mean(v) via 16 one-column PE matmuls per head into a [64, 8] PSUM
    tile; transposed by one identity matmul.  No big DVE reduce.
  * masks as mask-tile multiplies on the otherwise-idle DVE.
  * only 3 DMA queues exist (SP/ACT HWDGE + Pool SWDGE); loads are
    round-robined and issued first, the ACT exp-table warmup runs inside
    the initial DMA-latency dead window.
  * stores: band rows as 4-head-group DMAs that skip row 255; mean rows
    via seed stores (rows 255+2047) + DRAM->DRAM broadcasts of rows
    256..2046 on the same queue (ordered), split in two halves so the
    first half overlaps compute.
"""

import numpy as np
from contextlib import ExitStack

import concourse.bass as bass
import concourse.mybir as mybir
import concourse.tile as tile
from concourse.bass_utils import run_bass_kernel_spmd

B, H, L, D = 4, 16, 2048, 64
BH = B * H
NCORES = 8
PER = BH // NCORES     # 8 heads per core
BAND = 256             # band keys 1792..2047
NQ = 255               # valid band queries 0..254
NBC = 7                # non-band 256-row chunks (7*256 = 1792 rows)
VW = NBC * 2 * D + 2 * D   # 1024 floats per partition per head

F32 = mybir.dt.float32
F32R = mybir.dt.float32r
EXP = mybir.ActivationFunctionType.Exp
SCALE = 0.125          # 1/sqrt(D)
INVL = 1.0 / float(L)  # 2^-11, exact


def _build_bass():
    nc = bass.Bass()
    # qk: [128, 4*512]; heads even on partitions 0:64, odd on 64:128;
    # per head-slot 512 cols = [qT(256) | k0T(128) | k1T(128)]
    qk = nc.declare_dram_parameter("qk", [128, 4 * 512], F32R, isOutput=False)
    # vv: per head [128, 1024]: cols 0:896 non-band rows in pair layout
    # (c7, e2, d), cols 896:1024 band rows key-on-partition (c2, d)
    vv = nc.declare_dram_parameter("vv", [PER, 128, VW], F32, isOutput=False)
    out = nc.declare_dram_parameter("out", [PER, L, D], F32, isOutput=True)

    with tile.TileContext(nc) as tc:
        with ExitStack() as ctx:
            const = ctx.enter_context(tc.tile_pool(name="const", bufs=1))
            vpool = ctx.enter_context(tc.tile_pool(name="vpool", bufs=PER))
            io = ctx.enter_context(tc.tile_pool(name="io", bufs=1))
            ep = ctx.enter_context(tc.tile_pool(name="ep", bufs=3))
            sm = ctx.enter_context(tc.tile_pool(name="sm", bufs=8))
            ps_st = ctx.enter_context(tc.tile_pool(name="ps_st", bufs=2,
                                                   space="PSUM"))
            ps_u = ctx.enter_context(tc.tile_pool(name="ps_u", bufs=4,
                                                  space="PSUM"))
            ps_m = ctx.enter_context(tc.tile_pool(name="ps_m", bufs=1,
                                                  space="PSUM"))

            # ---- constants (Pool, first so the ACT warmup can start) ----
            scrap = const.tile([1, 2], F32, tag="scrap")
            nc.gpsimd.memset(scrap, 0.0)
            ones = const.tile([128, 1], F32, tag="ones")
            nc.gpsimd.memset(ones, 1.0)
            lones = const.tile([128, 1], F32, tag="lones")
            nc.gpsimd.memset(lones, INVL)
            ident = const.tile([64, 64], F32, tag="ident")
            nc.gpsimd.memset(ident, 1.0)
            nc.gpsimd.affine_select(
                out=ident, in_=ident, compare_op=mybir.AluOpType.is_equal,
                fill=0.0, base=0, channel_multiplier=1, pattern=[[-1, 64]],
            )
            # mask[p, f] = 1 iff p >= f + 1 (strict lower triangle)
            mask = const.tile([128, 128], F32, tag="mask")
            nc.gpsimd.memset(mask, 1.0)
            nc.gpsimd.affine_select(
                out=mask, in_=mask, compare_op=mybir.AluOpType.is_ge,
                fill=0.0, base=-1, channel_multiplier=1, pattern=[[-1, 128]],
            )
            # ACT exp-table warmup inside the DMA-latency dead window
            warm = const.tile([1, 2], F32, tag="warm")
            nc.scalar.activation(warm, scrap, EXP)

            # ---- loads (qk first; vv halves round-robined) ----
            qkt = io.tile([128, 4 * 512], F32R, tag="qkt")
            nc.sync.dma_start(out=qkt[:, 0:512], in_=qk[:, 0:512])
            nc.gpsimd.dma_start(out=qkt[:, 512:1024], in_=qk[:, 512:1024])
            nc.sync.dma_start(out=qkt[:, 1024:1536], in_=qk[:, 1024:1536])
            nc.gpsimd.dma_start(out=qkt[:, 1536:2048], in_=qk[:, 1536:2048])
            vts = []
            for h in range(PER):
                vt = vpool.tile([128, VW], F32, tag="vt")
                vts.append(vt)
            # (head, half) -> queue; SP x7, Pool x6, ACT x3; A-half (h0-3)
            # and B-half (h4-7) interleaved so both mean halves gate early
            HALF_Q = [
                (0, 0, "sp"), (0, 1, "pool"), (1, 0, "act"),
                (1, 1, "sp"), (2, 0, "pool"), (2, 1, "sp"),
                (3, 0, "pool"), (3, 1, "act"), (4, 0, "sp"),
                (4, 1, "pool"), (5, 0, "sp"), (5, 1, "sp"),
                (6, 0, "pool"), (6, 1, "sp"), (7, 0, "pool"),
                (7, 1, "act"),
            ]
            ENG = {"sp": nc.sync, "act": nc.scalar, "pool": nc.gpsimd}
            for h, half, qn in HALF_Q:
                sl = slice(half * 512, (half + 1) * 512)
                ENG[qn].dma_start(out=vts[h][:, sl], in_=vv[h, :, sl])

            # ---- per-head compute ----
            mean_ps = ps_m.tile([64, PER], F32, tag="mt")
            ob = io.tile([128, PER * 2 * D], F32, tag="ob")

            for h in range(PER):
                vt = vts[h]
                v0 = vt[:, 896:896 + D]
                v1 = vt[:, 896 + D:896 + 2 * D]

                # mean chain: 16 one-column matmuls into mean_ps[:, h]
                mcol = mean_ps[:, h:h + 1]
                for s in range(NBC * 2):
                    nc.tensor.matmul(mcol, lhsT=vt[:, s * D:(s + 1) * D],
                                     rhs=lones, start=(s == 0), stop=False)
                nc.tensor.matmul(mcol, lhsT=v0, rhs=lones,
                                 start=False, stop=False)
                nc.tensor.matmul(mcol, lhsT=v1, rhs=lones,
                                 start=False, stop=True)

                if h == 3 or h == 7:
                    # mean seed + broadcast for the finished half (emitted
                    # before this head's attention so the ident-matmul is
                    # not queued behind its U matmuls on PE)
                    g0 = h - 3
                    mt_sb = sm.tile([64, 4], F32, tag="mt_sb")
                    nc.vector.tensor_scalar_mul(mt_sb, mean_ps[:, g0:g0 + 4],
                                                1.0)
                    mrow_ps = ps_m.tile([4, 64], F32, tag="mrow")
                    nc.tensor.matmul(mrow_ps, lhsT=mt_sb, rhs=ident,
                                     start=True, stop=True)
                    mrow = sm.tile([4, 64], F32, tag="mrow_sb")
                    nc.vector.tensor_scalar_mul(mrow, mrow_ps, 1.0)
                    seed_in = bass.AP(
                        tensor=mrow.tensor, offset=mrow.offset,
                        ap=[list(mrow.ap[0]), [0, 2], [1, D]],
                    )
                    seed_out = bass.AP(
                        tensor=out, offset=g0 * L * D + NQ * D,
                        ap=[[L * D, 4], [(L - 1 - NQ) * D, 2], [1, D]],
                    )
                    bc_in = bass.AP(
                        tensor=out, offset=g0 * L * D + (L - 1) * D,
                        ap=[[0, L - 2 - NQ], [L * D, 4], [1, D]],
                    )
                    bc_out = bass.AP(
                        tensor=out, offset=g0 * L * D + (NQ + 1) * D,
                        ap=[[D, L - 2 - NQ], [L * D, 4], [1, D]],
                    )
                    if h == 3:
                        nc.gpsimd.dma_start(out=seed_out, in_=seed_in)
                        nc.gpsimd.dma_start(out=bc_out, in_=bc_in)
                    else:
                        # last half: per-head SBUF-replicated broadcasts,
                        # spread across queues (shorter serial tail)
                        for j in range(4):
                            bin_j = bass.AP(
                                tensor=mrow.tensor,
                                offset=mrow.offset + j * mrow.ap[0][0],
                                ap=[[mrow.ap[0][0], 1], [0, L - NQ], [1, D]],
                            )
                            bout_j = bass.AP(
                                tensor=out,
                                offset=(4 + j) * L * D + NQ * D,
                                ap=[[D, L - NQ], [1, D]],
                            )
                            eng = (nc.gpsimd, nc.scalar, nc.gpsimd,
                                   nc.sync)[j]
                            eng.dma_start(out=bout_j, in_=bin_j)

                base_p = (h % 2) * 64
                col0 = (h // 2) * 512
                qT = qkt[base_p:base_p + 64, col0:col0 + 256]
                k0 = qkt[base_p:base_p + 64, col0 + 256:col0 + 384]
                k1 = qkt[base_p:base_p + 64, col0 + 384:col0 + 512]

                # scores: keys on partitions; cols = [c0 x q0:128 | c1 x q0:256]
                st = ps_st.tile([128, 384], F32, tag="st")
                nc.tensor.matmul(st[:, 0:128], lhsT=k0, rhs=qT[:, 0:128],
                                 start=True, stop=True)
                nc.tensor.matmul(st[:, 128:384], lhsT=k1, rhs=qT,
                                 start=True, stop=True)

                # exp (|s|/8 <= ~6, no max subtraction needed)
                e = ep.tile([128, 384], F32, tag="e")
                nc.scalar.activation(e, st, EXP, scale=SCALE)
                # masks: strict-lower-triangle; DVE for early heads, Pool
                # (idle after its loads) for late heads
                if h < 6:
                    nc.vector.tensor_tensor(
                        out=e[:, 0:128], in0=e[:, 0:128], in1=mask,
                        op=mybir.AluOpType.mult)
                    nc.vector.tensor_tensor(
                        out=e[:, 256:384], in0=e[:, 256:384], in1=mask,
                        op=mybir.AluOpType.mult)
                else:
                    for sl in (slice(0, 128), slice(256, 384)):
                        nc.gpsimd.affine_select(
                            out=e[:, sl], in_=e[:, sl],
                            compare_op=mybir.AluOpType.is_ge,
                            fill=0.0, base=-1, channel_multiplier=1,
                            pattern=[[-1, 128]],
                        )

                # U = P^T V, den = P^T 1
                u0 = ps_u.tile([128, D + 1], F32, tag="u")
                nc.tensor.matmul(u0[:, 0:D], lhsT=e[:, 0:128], rhs=v0,
                                 start=True, stop=False)
                nc.tensor.matmul(u0[:, 0:D], lhsT=e[:, 128:256], rhs=v1,
                                 start=False, stop=True)
                nc.tensor.matmul(u0[:, D:D + 1], lhsT=e[:, 0:128], rhs=ones,
                                 start=True, stop=False)
                nc.tensor.matmul(u0[:, D:D + 1], lhsT=e[:, 128:256], rhs=ones,
                                 start=False, stop=True)
                u1 = ps_u.tile([128, D + 1], F32, tag="u")
                nc.tensor.matmul(u1[:, 0:D], lhsT=e[:, 256:384], rhs=v1,
                                 start=True, stop=True)
                nc.tensor.matmul(u1[:, D:D + 1], lhsT=e[:, 256:384], rhs=ones,
                                 start=True, stop=True)

                # normalize on DVE (recips free, muls 192)
                r0 = sm.tile([128, 1], F32, tag="r")
                r1 = sm.tile([128, 1], F32, tag="r")
                den1 = sm.tile([128, 1], F32, tag="r")
                nc.vector.reciprocal(r0, u0[:, D:D + 1])
                # query 255 (u1 row 127) is fully masked -> den 0; keep finite
                nc.vector.tensor_scalar_add(den1, u1[:, D:D + 1], 1e-30)
                nc.vector.reciprocal(r1, den1)
                nc.vector.tensor_scalar_mul(
                    ob[:, (2 * h) * D:(2 * h + 1) * D], u0[:, 0:D], r0)
                nc.vector.tensor_scalar_mul(
                    ob[:, (2 * h + 1) * D:(2 * h + 2) * D], u1[:, 0:D], r1)

                if h == 3 or h == 7:
                    # band stores for the finished 4-head group: rows 0..127
                    # (c0) and 128..254 (c1, partitions 0..126); row 255 is
                    # left to the mean path
                    g0 = h - 3
                    src = ob.rearrange("p (h c d) -> p h c d", h=PER, d=D)
                    inA = bass.AP(
                        tensor=src.tensor, offset=src.offset + g0 * 2 * D,
                        ap=[list(src.ap[0]), [2 * D, 4], [1, D]],
                    )
                    dstA = bass.AP(
                        tensor=out, offset=g0 * L * D,
                        ap=[[D, 128], [L * D, 4], [1, D]],
                    )
                    inB = bass.AP(
                        tensor=src.tensor,
                        offset=src.offset + g0 * 2 * D + D,
                        ap=[[src.ap[0][0], 127], [2 * D, 4], [1, D]],
                    )
                    dstB = bass.AP(
                        tensor=out, offset=g0 * L * D + 128 * D,
                        ap=[[D, 127], [L * D, 4], [1, D]],
                    )
                    engA = nc.sync
                    engB = nc.sync if h == 3 else nc.scalar
                    engA.dma_start(out=dstA, in_=inA)
                    engB.dma_start(out=dstB, in_=inB)


    return nc


def _legalize_waits(nc):
    """This walrus build rejects instructions carrying more than one attached
    sync wait: split them into preceding single-wait NoOps (same queue =>
    same semantics)."""
    n = 0
    for fn in nc.m.functions:
        for blk in fn.blocks:
            new_insts = []
            for inst in blk.instructions:
                si = inst.sync_info
                if si is not None and si.on_wait and len(si.on_wait) > 1:
                    for w in si.on_wait:
                        n += 1
                        new_insts.append(mybir.InstNoOp(
                            name=f"legwait-{n}",
                            engine=inst.engine,
                            ins=[], outs=[],
                            sync_info=mybir.SyncInfo(on_wait=[w], on_update=[]),
                            bass_nofuse=True,
                        ))
                    inst.sync_info = mybir.SyncInfo(
                        on_wait=[], on_update=list(si.on_update or []))
                new_insts.append(inst)
            blk.instructions[:] = new_insts


_NC = None
_LEGALIZED = False


def _get_nc(legalize=False):
    global _NC, _LEGALIZED
    if _NC is None:
        _NC = _build_bass()
    if legalize and not _LEGALIZED:
        _legalize_waits(_NC)
        _LEGALIZED = True
    return _NC


def _make_in_maps(q, k, v):
    qf = np.asarray(q, dtype=np.float32).reshape(BH, L, D)
    kf = np.asarray(k, dtype=np.float32).reshape(BH, L, D)
    vf = np.asarray(v, dtype=np.float32).reshape(BH, L, D)
    qT = qf[:, 0:BAND, :].transpose(0, 2, 1)          # [BH, 64, 256]
    kT = kf[:, L - BAND:L, :].transpose(0, 2, 1)      # [BH, 64, 256]
    qkcat = np.concatenate([qT, kT], axis=2)          # [BH, 64, 512]
    # vv: non-band pair layout cols 0:896, band key-on-partition cols 896:1024
    vvh = np.empty((BH, 128, VW), np.float32)
    vvh[:, :, 0:NBC * 2 * D] = (
        vf[:, 0:NBC * 256].reshape(BH, NBC, 128, 2, D)
        .transpose(0, 2, 1, 3, 4).reshape(BH, 128, NBC * 2 * D))
    vvh[:, :, NBC * 2 * D:] = (
        vf[:, L - BAND:L].reshape(BH, 2, 128, D)
        .transpose(0, 2, 1, 3).reshape(BH, 128, 2 * D))
    in_maps = []
    for c in range(NCORES):
        s = slice(c * PER, (c + 1) * PER)
        qkc = qkcat[s]                                 # [8, 64, 512]
        qkp = np.empty((128, 4 * 512), np.float32)
        qkp[0:64] = qkc[0::2].transpose(1, 0, 2).reshape(64, 4 * 512)
        qkp[64:128] = qkc[1::2].transpose(1, 0, 2).reshape(64, 4 * 512)
        in_maps.append({
            "qk": np.ascontiguousarray(qkp),
            "vv": np.ascontiguousarray(vvh[s]),
        })
    return in_maps


def _run(q, k, v, **kwargs):
    nc = _get_nc(legalize=True)
    in_maps = _make_in_maps(q, k, v)
    return run_bass_kernel_spmd(nc, in_maps, list(range(NCORES)), **kwargs)


def kernel(q, k, v):
    res = _run(q, k, v)
    outs = [res.results[c]["out"] for c in range(NCORES)]
    return np.concatenate(outs, axis=0).reshape(B, H, L, D)
